# revision 1
# baseline (speedup 1.0000x reference)
"""Multi-head attention Bass kernel for Trainium2, sharded over 8 NeuronCores.

Problem: x [2, 2048, 1024] fp32; W_qkv [3072, 1024]; b_qkv [3072].
  qkv = x @ W_qkv.T + b_qkv ; split into Q,K,V of 8 heads x 128 dims;
  out  = softmax(Q K^T / sqrt(128)) V, heads re-concatenated -> [2, 2048, 1024].

Sharding: 16 (batch, head) pairs over 8 cores -> each core owns one batch
slice (b = core//4) and two heads (h0 = 2*(core%4), h0+1). Each core gets
its batch's x slice [2048, 1024] plus the W^T/bias columns for its heads,
computes the projection and full non-causal attention for its two heads,
and returns [2048, 256] (the two heads' output columns). No collectives.

Kernel internals (per core):
 - x is cast to bf16 and bounced through DRAM so the DMA-transpose engine
   can produce x^T (contraction dim on partitions) for the projection.
 - Q^T, K^T produced directly in [dh, tok] layout (bias added during the
   PSUM->SBUF copy); V in natural [tok, dh] layout with a ones column
   appended so the P@V matmul also produces softmax denominators.
 - Scores are computed transposed (S^T tiles [k, q]) so exp(S^T) is
   directly the lhsT for the P@V matmul -- no on-chip transposes of the
   4M-element attention matrix. exp is computed without max subtraction
   (scores are O(1) here; exp is safely in fp32/bf16 range).
 - Final output = (P@V) * 1/denominator, written as fp32.
"""

import math
from contextlib import ExitStack

import numpy as np

import concourse.bass as bass
import concourse.tile as tile
from concourse import bacc, mybir
from concourse.bass_utils import run_bass_kernel_spmd

# Problem constants (hardcoded per the harness contract).
B = 2
S = 2048
D = 1024
H = 8
DH = 128
N_CORES = 8
HPC = 2  # heads per core
SC = S  # tokens per core (one full batch element)
WCOLS = 3 * HPC * DH  # 768: [q0 q1 k0 k1 v0 v1] blocks of 128
SCALE = 1.0 / math.sqrt(DH)

F32 = mybir.dt.float32
BF16 = mybir.dt.bfloat16

N_CHUNK = 4  # token chunks for the x-transpose pipeline
CHUNK = SC // N_CHUNK  # 512
KO = D // 128  # 8 contraction chunks


def _mha_body(ctx: ExitStack, tc: tile.TileContext, out, x, wt, bias):
    nc = tc.nc

    consts = ctx.enter_context(tc.tile_pool(name="consts", bufs=1))
    xtp = ctx.enter_context(tc.tile_pool(name="xtp", bufs=1))
    qkvp = ctx.enter_context(tc.tile_pool(name="qkvp", bufs=1))

    # ---- constants: W^T (bf16 from host) and biases ----
    # V-projection weight columns load first (earliest consumer), then x^T
    # chunks, then Q/K weight columns -- matches first-use order on the PE.
    # hoist the ACT exp table load (~2.7us) to kernel start, under the input DMA
    warm = consts.tile([128, 1], F32)
    nc.vector.memset(warm, 0.0)
    nc.scalar.activation(warm, warm, mybir.ActivationFunctionType.Exp)

    wt_sb = consts.tile([128, KO, WCOLS], BF16)
    wtr = wt.rearrange("(ko ki) m -> ki ko m", ki=128)
    nc.sync.dma_start(wt_sb[:, :, 2 * HPC * DH:], wtr[:, :, 2 * HPC * DH:])  # V cols

    # xT chunks: [128 d_inner, KO d_outer, CHUNK tokens] bf16 (host pre-transposed)
    xt = [xtp.tile([128, KO, CHUNK], BF16, tag=f"xt{c}", name=f"xt{c}") for c in range(N_CHUNK)]
    xr = x.rearrange("(ko p) t -> p ko t", p=128)
    for c in range(N_CHUNK):
        nc.sync.dma_start(xt[c], xr[:, :, c * CHUNK:(c + 1) * CHUNK])

    nc.sync.dma_start(wt_sb[:, :, :2 * HPC * DH], wtr[:, :, :2 * HPC * DH])  # Q/K cols

    # per-partition bias tiles for Q^T / K^T copies: q_h at h*128, k_h at 256+h*128
    bqk = []
    for i in range(2 * HPC):  # q0 q1 k0 k1
        bt = consts.tile([128, 1], F32, tag=f"bqk{i}")
        nc.sync.dma_start(bt, bias[i * 128:(i + 1) * 128].rearrange("(p o) -> p o", o=1))
        bqk.append(bt)
    # V bias replicated across partitions [128, 256]
    bv_rep = consts.tile([128, HPC * DH], F32)
    nc.gpsimd.dma_start(bv_rep, bias[2 * HPC * DH:][None, :].to_broadcast([128, HPC * DH]))

    # ---- persistent QKV tiles ----
    qT = qkvp.tile([128, HPC, SC], BF16, tag="qT")  # [dh, h, tok]
    kT = qkvp.tile([128, HPC, SC], BF16, tag="kT")
    v_sb = qkvp.tile([128, HPC, SC // 128, DH + 1], BF16, tag="v")  # [tok_i, h, tok_o, dh+1]
    nc.vector.memset(v_sb[:, :, :, DH:DH + 1], 1.0)

    # ---- projection + attention, per head interleaved ----
    QB = 256  # query block width (2 PV accumulators = 2 PSUM banks)
    NQB = SC // QB
    NKT = SC // 128  # 16 key tiles
    KPS = 4  # key tiles per score tile (exp runs on [128, KPS*QB] = [128,1024])
    proj_ps = ctx.enter_context(tc.tile_pool(name="proj_ps", bufs=2, space="PSUM"))
    st_ps = ctx.enter_context(tc.tile_pool(name="st_ps", bufs=2, space="PSUM"))
    pv_ps = ctx.enter_context(tc.tile_pool(name="pv_ps", bufs=2, space="PSUM"))
    atp = ctx.enter_context(tc.tile_pool(name="atp", bufs=3))
    outp = ctx.enter_context(tc.tile_pool(name="outp", bufs=2))
    rcp = ctx.enter_context(tc.tile_pool(name="rcp", bufs=8))

    def emit_qk(h):
        # Q^T and K^T for this head: out [128 dh, tok], lhsT = W^T chunk
        for qk in range(2):
            dst = qT if qk == 0 else kT
            col = qk * HPC * DH + h * DH
            for tb in range(N_CHUNK):
                ps = proj_ps.tile([128, CHUNK], F32, tag="ps", name="ps")
                for ko in range(KO):
                    nc.tensor.matmul(
                        ps,
                        lhsT=wt_sb[:, ko, col:col + DH],
                        rhs=xt[tb][:, ko, :],
                        start=(ko == 0),
                        stop=(ko == KO - 1),
                    )
                nc.vector.tensor_scalar_add(
                    dst[:, h, tb * CHUNK:(tb + 1) * CHUNK], ps, bqk[qk * HPC + h]
                )

    def emit_v():
        # V (both heads fused): out [128 tok, 2*dh], lhsT = x^T chunk
        for tb in range(SC // 128):
            ps = proj_ps.tile([128, CHUNK], F32, tag="ps", name="ps")
            psv = ps[:, :HPC * DH]
            for ko in range(KO):
                nc.tensor.matmul(
                    psv,
                    lhsT=xt[tb // 4][:, ko, (tb % 4) * 128:(tb % 4 + 1) * 128],
                    rhs=wt_sb[:, ko, 2 * HPC * DH:],
                    start=(ko == 0),
                    stop=(ko == KO - 1),
                )
            nc.vector.tensor_add(
                v_sb[:, :, tb, 0:DH],
                psv.rearrange("p (h d) -> p h d", h=HPC),
                bv_rep.rearrange("p (h d) -> p h d", h=HPC),
            )

    def emit_attn(h):
        for qb in range(NQB):
            qs = slice(qb * QB, (qb + 1) * QB)
            pvs = [pv_ps.tile([128, DH + 1], F32, tag="pv", name=f"pv{j}") for j in range(QB // 128)]
            for ktg in range(NKT // KPS):
                st = st_ps.tile([128, KPS, QB], F32, tag="st", name="st")
                for i in range(KPS):
                    kt = ktg * KPS + i
                    nc.tensor.matmul(
                        st[:, i, :],
                        lhsT=kT[:, h, kt * 128:(kt + 1) * 128],
                        rhs=qT[:, h, qs],
                        start=True,
                        stop=True,
                    )
                at = atp.tile([128, KPS, QB], BF16, tag="at", name="at")
                nc.scalar.activation(at, st, mybir.ActivationFunctionType.Exp, scale=SCALE)
                for i in range(KPS):
                    kt = ktg * KPS + i
                    for j in range(QB // 128):
                        nc.tensor.matmul(
                            pvs[j],
                            lhsT=at[:, i, j * 128:(j + 1) * 128],
                            rhs=v_sb[:, h, kt, :],
                            start=(kt == 0),
                            stop=(kt == NKT - 1),
                        )
            ot = outp.tile([128, QB // 128, DH], F32, tag="ot", name="ot")
            for j in range(QB // 128):
                rc = rcp.tile([128, 1], F32, tag="rc", name="rc")
                nc.vector.reciprocal(rc, pvs[j][:, DH:DH + 1])
                nc.vector.tensor_scalar_mul(ot[:, j, :], pvs[j][:, 0:DH], rc)
            nc.sync.dma_start(
                out[qs, h * DH:(h + 1) * DH].rearrange("(j p) c -> p j c", p=128),
                ot,
            )

    emit_v()
    emit_qk(0)
    emit_attn(0)
    emit_qk(1)
    emit_attn(1)


def build_program():
    nc = bacc.Bacc("TRN2", target_bir_lowering=False, debug=False)
    x = nc.dram_tensor("x", [D, SC], BF16, kind="ExternalInput").ap()
    wt = nc.dram_tensor("wt", [D, WCOLS], BF16, kind="ExternalInput").ap()
    bias = nc.dram_tensor("bias", [WCOLS], F32, kind="ExternalInput").ap()
    out = nc.dram_tensor("out", [SC, HPC * DH], F32, kind="ExternalOutput").ap()
    with tile.TileContext(nc) as tc:
        with ExitStack() as ctx:
            _mha_body(ctx, tc, out, x, wt, bias)
    nc.compile()
    return nc


_NC = None


def _get_nc():
    global _NC
    if _NC is None:
        _NC = build_program()
    return _NC


def make_in_maps(x, W_qkv, b_qkv):
    import ml_dtypes

    x = np.asarray(x, dtype=np.float32)
    W = np.asarray(W_qkv, dtype=np.float32)
    b = np.asarray(b_qkv, dtype=np.float32)
    x_bf = x.astype(ml_dtypes.bfloat16)
    in_maps = []
    for c in range(N_CORES):
        bsel = c // 4
        h0 = HPC * (c % 4)
        rows = np.concatenate(
            [qkv * D + np.arange(h0 * DH, (h0 + HPC) * DH) for qkv in range(3)]
        )
        Wc = W[rows]  # [768, 1024]
        in_maps.append(
            {
                "x": np.ascontiguousarray(x_bf[bsel].T),
                "wt": np.ascontiguousarray(Wc.T.astype(ml_dtypes.bfloat16)),
                "bias": np.ascontiguousarray(b[rows]),
            }
        )
    return in_maps


def gather_output(results):
    outp = np.empty((B, S, D), np.float32)
    for c in range(N_CORES):
        o = results[c]["out"]
        bsel = c // 4
        h0 = HPC * (c % 4)
        outp[bsel, :, h0 * DH:(h0 + HPC) * DH] = o
    return outp


def kernel(x, W_qkv, b_qkv, **run_kwargs):
    in_maps = make_in_maps(x, W_qkv, b_qkv)
    res = run_bass_kernel_spmd(_get_nc(), in_maps, core_ids=list(range(N_CORES)), **run_kwargs)
    out = gather_output(res.results)
    if run_kwargs:
        kernel.last_result = res
    return out



# revision 3
# speedup vs baseline: 1.1744x; 1.1744x over previous
"""Multi-head attention Bass kernel for Trainium2, sharded over 8 NeuronCores.

Problem: x [2, 2048, 1024] fp32; W_qkv [3072, 1024]; b_qkv [3072].
  qkv = x @ W_qkv.T + b_qkv ; split into Q,K,V of 8 heads x 128 dims;
  out  = softmax(Q K^T / sqrt(128)) V, heads re-concatenated -> [2, 2048, 1024].

Sharding: 16 (batch, head) pairs over 8 cores -> each core owns one batch
slice (b = core//4) and two heads (h0 = 2*(core%4), h0+1). Each core gets
its batch's x slice [2048, 1024] plus the W^T/bias columns for its heads,
computes the projection and full non-causal attention for its two heads,
and returns [2048, 256] (the two heads' output columns). No collectives.

v2 (startup + overlap optimized):
 - Host pre-packs x^T as [128, 8, 2048] so each DMA descriptor is an 8KB
   contiguous run; x streams in as 4 ko-pair chunks that feed a chunk-gated
   K0/Q0 projection (8 PSUM banks, ko-outer) so the PE starts ~5us earlier.
 - Weights arrive as two fully-contiguous tensors: wta = [K0|Q0] columns
   (needed first), wtb = [K1|Q1|V0|V1].
 - V projection is split per head: V0 runs before attention of head 0;
   V1-proj matmuls are interleaved into head 1's attention loop as PE
   filler while the ACT engine grinds the exps (attn is ACT-paced there).
 - Scores for (head0, qb0) are emitted right after the K0/Q0 drain so the
   ACT exp stream starts as early as possible.
 - Same math as v1: scores computed transposed (S^T tiles [k, q]); exp on
   ACT (scale folded in, no max subtraction -- scores are O(1)); PV matmul
   with stationary P-chunks and a ones column appended to V so softmax
   denominators fall out of the same matmuls; final scale by 1/denom on DVE.
"""

import math
from contextlib import ExitStack

import numpy as np

import concourse.bass as bass
import concourse.tile as tile
from concourse import bacc, mybir
from concourse.bass_utils import run_bass_kernel_spmd

# Problem constants (hardcoded per the harness contract).
B = 2
S = 2048
D = 1024
H = 8
DH = 128
N_CORES = 8
HPC = 2  # heads per core
SC = S  # tokens per core (one full batch element)
SCALE = 1.0 / math.sqrt(DH)

F32 = mybir.dt.float32
BF16 = mybir.dt.bfloat16

KO = D // 128  # 8 contraction chunks
NXCH = 4  # x arrives in 4 ko-pair chunks
QB = 256  # query block width
NQB = SC // QB  # 8
NKT = S // 128  # 16 key tiles
KPS = 4  # key tiles per score/exp group (exp on [128, KPS*QB] = [128,1024])
NTB = 4  # 512-token tiles for Q/K projection
TB = SC // NTB  # 512


def _mha_body(ctx: ExitStack, tc: tile.TileContext, out, x, wta, wtb, bias):
    nc = tc.nc

    consts = ctx.enter_context(tc.tile_pool(name="consts", bufs=1))
    xtp = ctx.enter_context(tc.tile_pool(name="xtp", bufs=1))
    qkvp = ctx.enter_context(tc.tile_pool(name="qkvp", bufs=1))

    # hoist the ACT exp table load (~2.7us) to kernel start, under the input DMA
    warm = consts.tile([128, 1], F32)
    nc.vector.memset(warm, 0.0)
    nc.scalar.activation(warm, warm, mybir.ActivationFunctionType.Exp)

    # ---- bias tiles (tiny, DMA'd first) ----
    # host bias layout: [q0 q1 k0 k1 | v0 v1]; per-partition tiles for Q/K
    bqk = []
    for i in range(2 * HPC):  # q0 q1 k0 k1
        bt = consts.tile([128, 1], F32, tag=f"bqk{i}")
        nc.sync.dma_start(bt, bias[i * 128:(i + 1) * 128].rearrange("(p o) -> p o", o=1))
        bqk.append(bt)
    # V bias replicated across partitions [128, 256] (both heads)
    bv_rep = consts.tile([128, HPC * DH], F32)
    nc.gpsimd.dma_start(bv_rep, bias[2 * HPC * DH:][None, :].to_broadcast([128, HPC * DH]))

    # ---- input DMAs: wta (K0|Q0), x chunks, wtb (K1|Q1|V0|V1) ----
    wta_sb = consts.tile([128, KO, 2 * DH], BF16)  # [ki, ko, m]: m = [k0 q0]
    nc.sync.dma_start(wta_sb, wta)

    xt = xtp.tile([128, KO, SC], BF16)  # [ki, ko, tok]
    xch = []
    for c in range(NXCH):
        d = nc.sync.dma_start(xt[:, 2 * c:2 * c + 2, :], x[:, 2 * c:2 * c + 2, :])
        xch.append(d)

    wtb_sb = consts.tile([128, KO, 4 * DH], BF16)  # m = [k1 q1 v0 v1]
    nc.sync.dma_start(wtb_sb, wtb)

    # ---- persistent QKV tiles ----
    qT = qkvp.tile([128, HPC, SC], BF16, tag="qT")  # [dh, h, tok]
    kT = qkvp.tile([128, HPC, SC], BF16, tag="kT")
    v_sb = qkvp.tile([128, HPC, SC // 128, DH + 1], BF16, tag="v")  # [tok_i, h, tok_o, dh+1]
    nc.vector.memset(v_sb[:, :, :, DH:DH + 1], 1.0)

    # ---- phase 0: K0 + Q0 projection, ko-outer (chunk-gated), 8 PSUM banks ----
    with ExitStack() as ctx0:
        p0 = ctx0.enter_context(tc.tile_pool(name="p0ps", bufs=1, space="PSUM"))
        # 8 tiles of [128, 512] f32 = 8 banks: [kq][tb]
        p0t = [[p0.tile([128, TB], F32, tag=f"p0_{kq}_{tb}", name=f"p0_{kq}_{tb}")
                for tb in range(NTB)] for kq in range(2)]
        for c in range(NXCH):
            for ko in (2 * c, 2 * c + 1):
                for kq in range(2):  # 0 = k0, 1 = q0
                    for tb in range(NTB):
                        nc.tensor.matmul(
                            p0t[kq][tb],
                            lhsT=wta_sb[:, ko, kq * DH:(kq + 1) * DH],
                            rhs=xt[:, ko, tb * TB:(tb + 1) * TB],
                            start=(ko == 0),
                            stop=(ko == KO - 1),
                        )
        # drain with bias: k0 -> kT[:,0,:], q0 -> qT[:,0,:]; K first and then
        # Q's first block so scores(0, qb0) can issue as early as possible
        for tb in range(NTB):
            nc.vector.tensor_scalar_add(kT[:, 0, tb * TB:(tb + 1) * TB], p0t[0][tb], bqk[HPC + 0])
        nc.vector.tensor_scalar_add(qT[:, 0, 0:TB], p0t[1][0], bqk[0])
        for tb in range(1, NTB):
            nc.vector.tensor_scalar_add(qT[:, 0, tb * TB:(tb + 1) * TB], p0t[1][tb], bqk[0])

    # ---- main pools (reuse phase-0 PSUM banks; Tile inserts WAR syncs) ----
    proj_ps = ctx.enter_context(tc.tile_pool(name="proj_ps", bufs=2, space="PSUM"))
    st_ps = ctx.enter_context(tc.tile_pool(name="st_ps", bufs=2, space="PSUM"))
    pv_ps = ctx.enter_context(tc.tile_pool(name="pv_ps", bufs=2, space="PSUM"))
    atp = ctx.enter_context(tc.tile_pool(name="atp", bufs=3))
    outp = ctx.enter_context(tc.tile_pool(name="outp", bufs=2))
    rcp = ctx.enter_context(tc.tile_pool(name="rcp", bufs=8))

    def emit_qk1_group(tb):
        # K1/Q1 projection for one token block (ko-inner, proj pool)
        for kq in range(2):  # 0 = k1, 1 = q1
            ps = proj_ps.tile([128, TB], F32, tag="ps", name="ps")
            for ko in range(KO):
                nc.tensor.matmul(
                    ps,
                    lhsT=wtb_sb[:, ko, kq * DH:(kq + 1) * DH],
                    rhs=xt[:, ko, tb * TB:(tb + 1) * TB],
                    start=(ko == 0),
                    stop=(ko == KO - 1),
                )
            dst = kT if kq == 0 else qT
            b = bqk[HPC + 1] if kq == 0 else bqk[1]
            nc.vector.tensor_scalar_add(dst[:, 1, tb * TB:(tb + 1) * TB], ps, b)

    def emit_v_group(h, tb):
        # V projection for head h, one 128-token tile: out [128 tok, 128]
        ps = proj_ps.tile([128, TB], F32, tag="ps", name="ps")
        psv = ps[:, :DH]
        for ko in range(KO):
            nc.tensor.matmul(
                psv,
                lhsT=xt[:, ko, tb * 128:(tb + 1) * 128],
                rhs=wtb_sb[:, ko, (2 + h) * DH:(3 + h) * DH],
                start=(ko == 0),
                stop=(ko == KO - 1),
            )
        nc.vector.tensor_scalar_add(
            v_sb[:, h, tb, 0:DH], psv, bv_rep[:, h * DH:h * DH + 1]
        )

    def emit_scores(h, qb):
        qs = slice(qb * QB, (qb + 1) * QB)
        ats = []
        for ktg in range(NKT // KPS):
            st = st_ps.tile([128, KPS, QB], F32, tag="st", name="st")
            for i in range(KPS):
                kt = ktg * KPS + i
                nc.tensor.matmul(
                    st[:, i, :],
                    lhsT=kT[:, h, kt * 128:(kt + 1) * 128],
                    rhs=qT[:, h, qs],
                    start=True,
                    stop=True,
                )
            at = atp.tile([128, KPS, QB], BF16, tag="at", name="at")
            nc.scalar.activation(at, st, mybir.ActivationFunctionType.Exp, scale=SCALE)
            ats.append(at)
        return ats

    def emit_pv(h, qb, ats):
        qs = slice(qb * QB, (qb + 1) * QB)
        pvs = [pv_ps.tile([128, DH + 1], F32, tag="pv", name=f"pv{j}") for j in range(QB // 128)]
        for ktg in range(NKT // KPS):
            at = ats[ktg]
            for i in range(KPS):
                kt = ktg * KPS + i
                for j in range(QB // 128):
                    nc.tensor.matmul(
                        pvs[j],
                        lhsT=at[:, i, j * 128:(j + 1) * 128],
                        rhs=v_sb[:, h, kt, :],
                        start=(kt == 0),
                        stop=(kt == NKT - 1),
                    )
        ot = outp.tile([128, QB // 128, DH], F32, tag="ot", name="ot")
        for j in range(QB // 128):
            rc = rcp.tile([128, 1], F32, tag="rc", name="rc")
            nc.vector.reciprocal(rc, pvs[j][:, DH:DH + 1])
            nc.vector.tensor_scalar_mul(ot[:, j, :], pvs[j][:, 0:DH], rc)
        nc.sync.dma_start(
            out[qs, h * DH:(h + 1) * DH].rearrange("(j p) c -> p j c", p=128),
            ot,
        )

    # ---- emission schedule ----
    # scores for (h0, qb0) immediately after phase-0 drain: starts ACT early
    ats00 = emit_scores(0, 0)
    # V0 projection (x fully resident by now in the pipeline)
    for tb in range(SC // 128):
        emit_v_group(0, tb)
    emit_pv(0, 0, ats00)
    # attn(0) qb1..7, interleaved with K1/Q1 projection token blocks
    qk1_tbs = list(range(NTB))
    for qb in range(1, NQB):
        ats = emit_scores(0, qb)
        if qk1_tbs:
            emit_qk1_group(qk1_tbs.pop(0))
        emit_pv(0, qb, ats)
    while qk1_tbs:
        emit_qk1_group(qk1_tbs.pop(0))
    # attn(1), interleaved with V1 projection (PE filler while ACT does exps);
    # V1 tiles for key tile kt must be projected before PV of any qb reaches
    # them, so front-load two V1 tiles per qb iteration.
    v1_tbs = list(range(SC // 128))
    for qb in range(NQB):
        ats = emit_scores(1, qb)
        nv = 2 if qb > 0 else 16  # qb0's PV needs all of V1
        for _ in range(nv):
            if v1_tbs:
                emit_v_group(1, v1_tbs.pop(0))
        emit_pv(1, qb, ats)


def build_program():
    nc = bacc.Bacc("TRN2", target_bir_lowering=False, debug=False)
    x = nc.dram_tensor("x", [128, KO, SC], BF16, kind="ExternalInput").ap()
    wta = nc.dram_tensor("wta", [128, KO, 2 * DH], BF16, kind="ExternalInput").ap()
    wtb = nc.dram_tensor("wtb", [128, KO, 4 * DH], BF16, kind="ExternalInput").ap()
    bias = nc.dram_tensor("bias", [3 * HPC * DH], F32, kind="ExternalInput").ap()
    out = nc.dram_tensor("out", [SC, HPC * DH], F32, kind="ExternalOutput").ap()
    with tile.TileContext(nc) as tc:
        with ExitStack() as ctx:
            _mha_body(ctx, tc, out, x, wta, wtb, bias)
    nc.compile()
    return nc


_NC = None


def _get_nc():
    global _NC
    if _NC is None:
        _NC = build_program()
    return _NC


def make_in_maps(x, W_qkv, b_qkv):
    import ml_dtypes

    x = np.asarray(x, dtype=np.float32)
    W = np.asarray(W_qkv, dtype=np.float32)
    b = np.asarray(b_qkv, dtype=np.float32)
    x_bf = x.astype(ml_dtypes.bfloat16)
    in_maps = []
    for c in range(N_CORES):
        bsel = c // 4
        h0 = HPC * (c % 4)
        # x^T as [ki=128, ko=8, tok]: element (p, ko, t) = x[bsel].T[ko*128+p, t]
        xT = np.ascontiguousarray(
            x_bf[bsel].T.reshape(KO, 128, SC).transpose(1, 0, 2)
        )
        # W rows for this core's heads: q_h at h*128, k_h at 256+h*128, v_h at 512+h*128
        def wrows(block, h):  # block: 0=q, 1=k, 2=v
            return W[block * 2 * DH + (h0 + h) * DH:block * 2 * DH + (h0 + h) * DH + DH]

        # wta m-order: [k0 q0]; wtb m-order: [k1 q1 v0 v1]
        wa = np.concatenate([wrows(1, 0), wrows(0, 0)], axis=0)  # [256, 1024]
        wb = np.concatenate([wrows(1, 1), wrows(0, 1), wrows(2, 0), wrows(2, 1)], axis=0)

        def pack_wt(wm):  # [m, 1024] -> [ki=128, ko=8, m]
            wt = wm.T.astype(ml_dtypes.bfloat16)  # [1024, m]
            return np.ascontiguousarray(wt.reshape(KO, 128, wm.shape[0]).transpose(1, 0, 2))

        # bias host order: [q0 q1 k0 k1 v0 v1] blocks of 128
        brows = np.concatenate([
            b[(h0 + 0) * DH:(h0 + 1) * DH],
            b[(h0 + 1) * DH:(h0 + 2) * DH],
            b[2 * DH + (h0 + 0) * DH:2 * DH + (h0 + 1) * DH],
            b[2 * DH + (h0 + 1) * DH:2 * DH + (h0 + 2) * DH],
            b[4 * DH + (h0 + 0) * DH:4 * DH + (h0 + 1) * DH],
            b[4 * DH + (h0 + 1) * DH:4 * DH + (h0 + 2) * DH],
        ])
        in_maps.append(
            {
                "x": xT,
                "wta": pack_wt(wa),
                "wtb": pack_wt(wb),
                "bias": np.ascontiguousarray(brows),
            }
        )
    return in_maps


def gather_output(results):
    outp = np.empty((B, S, D), np.float32)
    for c in range(N_CORES):
        o = results[c]["out"]
        bsel = c // 4
        h0 = HPC * (c % 4)
        outp[bsel, :, h0 * DH:(h0 + HPC) * DH] = o
    return outp


def kernel(x, W_qkv, b_qkv, **run_kwargs):
    in_maps = make_in_maps(x, W_qkv, b_qkv)
    res = run_bass_kernel_spmd(_get_nc(), in_maps, core_ids=list(range(N_CORES)), **run_kwargs)
    out = gather_output(res.results)
    if run_kwargs:
        kernel.last_result = res
    return out
